# revision 11
# baseline (speedup 1.0000x reference)
"""Trainium2 Bass kernel for nn_CoAttentionFusionBlock.

Math: the reference's softmax is over a singleton dim, so its weights are
exactly 1.0 and o1/o2 equal the raw features bit-for-bit. The module reduces to

    out = concat([feat_depth, feat_rgb], axis=1) @ W_f.T + b_f        # [B, D]

W_k1/b_k1/W_k2/b_k2 only feed the (dead) score path and are never needed.

Distribution: pure data parallel over the batch dim across 8 NeuronCores.
Each core computes yT = WfT.T @ xT with operands pre-transposed on host so the
contraction dim (2048) lands on SBUF partitions.

v3 design notes (each item traced on HW):
  * bf16 operands. fp32r ("fp32_mode=HIGH") disables the compiler's Fast
    Weight Load, leaving the PE LDWEIGHTS-chain bound at 226.7 ns per 512-col
    matmul. bf16 enables FWL: LDWEIGHTS drops 187->97 ns and the measured
    cadence hits the 216 ns fill floor (512 cols @ 2.4 GHz + issue). Accuracy
    2.3e-3 vs the fp32 reference (gate 2e-2). PSUM output must stay one fp32
    bank (512 cols) -- wider matmuls cross a PSUM bank boundary (ISA error).
  * x is packed ON HOST into SBUF layout [part, slab, ktile, 512] so every
    load DMA moves contiguous 2-16 KB per partition line. With the naive
    [K, B] layout the bf16 tiles had 1 KB lines and the single HWDGE queue
    saturated at ~133 GB/s (packet-rate bound), starving the PE for 52 us.
  * All loads are issued up front on the sync HWDGE queue (big lines make it
    HBM-bound, not packet-rate bound); no store dma_start precedes any load,
    so the in-order sequencer never head-of-line-blocks a load behind a
    DVE-gated store. (A variant driving the scalar HWDGE queue in parallel
    hit NRT_EXEC_UNIT_UNRECOVERABLE on HW -- avoided.)
  * Slab 0 runs t-outer/j-inner with all 8 PSUM banks open as independent
    accumulation groups: each arriving weight k-tile immediately unlocks 8
    matmuls (~1.7 us PE work vs ~1.3 us DMA), so the PE streams at full rate
    during the weight load instead of idling ~13 us (v1) behind it.
  * Slabs 1-7 run j-outer/t-inner, one 16-matmul group per PSUM bank,
    rotating. Eviction: DVE tensor_scalar_add (PSUM + per-partition bias ->
    SBUF fp32) then DMA. The final group evicts in 4x128 chunks (tail trim).

Measured (8-core SPMD, neuron-profile): 241.7 us vs 255.1 us for the v1
fp32r kernel. The PE stream is gapless at the 216 ns cadence floor
(1024 matmuls = 221 us); the rest is ~12 us fixed preamble/first-tiles and
~7 us store-drain + epilogue. Rel err 2.3e-3 (gate 2e-2).
"""

import numpy as np
import ml_dtypes

import concourse.bacc as bacc
import concourse.mybir as mybir
import concourse.tile as tile
from concourse.bass_utils import run_bass_kernel_spmd

B = 32768
D = 1024
NCORES = 8
BLOC = B // NCORES  # 4096 batch rows per core
K = 2 * D  # 2048 contraction dim
P = 128  # partitions
NT = 512  # moving free dim per matmul (one PSUM bank of fp32)
KT = K // P  # 16 k-tiles
JT = D // P  # 8 output-row tiles
ST = BLOC // NT  # 8 slabs of 512 batch cols
SLAB_ELEMS = KT * NT  # 8192 elems per partition per slab

FP32 = mybir.dt.float32
BF16 = mybir.dt.bfloat16
NP_BF16 = np.dtype(ml_dtypes.bfloat16)

# test.py can flip these to profile; harness leaves them alone.
TRACE = False
TRACE_DIR = None
LAST_RESULT = None
DT_IN = "bf16"  # informational; v3 is bf16-only


def _build_nc():
    # Bacc (not raw Bass): its compile() runs move_matmul_waits_to_ldweights +
    # generate_event_semaphores, which split sync waits to <=1 per instruction
    # (TRN2 HW limit -- raw Bass hits "Too many sync wait commands" in walrus).
    nc = bacc.Bacc(None)
    # xP is host-packed to SBUF order: xP[p, ((s*KT + t)*NT + b)]
    #   = concat(feat_depth, feat_rgb).T[t*P + p, s*NT + b]
    xP = nc.declare_dram_parameter("xP", [P, ST * KT * NT], BF16, isOutput=False)
    wT = nc.declare_dram_parameter("wT", [K, D], BF16, isOutput=False)
    biasT = nc.declare_dram_parameter("biasT", [P, JT], FP32, isOutput=False)
    yT = nc.declare_dram_parameter("yT", [D, BLOC], FP32, isOutput=True)

    wT_v = wT.rearrange("(t p) j -> p t j", p=P)  # [128, KT, D], 2 KB lines
    yT_v = yT.rearrange("(j p) b -> j p b", p=P)  # [JT, 128, BLOC]

    with tile.TileContext(nc) as tc:
        with (
            tc.tile_pool(name="wpool", bufs=1) as wpool,
            tc.tile_pool(name="xpool", bufs=1) as xpool,
            tc.tile_pool(name="opool", bufs=6) as opool,
            tc.tile_pool(name="bpool", bufs=1) as bpool,
            tc.tile_pool(name="psum", bufs=8, space="PSUM") as psum_pool,
        ):
            bias_sb = bpool.tile([P, JT], FP32)
            nc.sync.dma_start(out=bias_sb[:], in_=biasT[:, :])

            w_sb = wpool.tile([P, KT * D], BF16)
            x_sb = [
                xpool.tile([P, SLAB_ELEMS], BF16, tag=f"x{s}", name=f"x_sb{s}")
                for s in range(ST)
            ]

            def load_slab(eng, s):
                eng.dma_start(
                    out=x_sb[s][:],
                    in_=xP[:, s * SLAB_ELEMS : (s + 1) * SLAB_ELEMS],
                )

            # All loads on the sync HWDGE queue. With 2-16 KB contiguous
            # lines the queue is HBM-bound (~358 GB/s), not packet-rate
            # bound, so one queue suffices. Startup stream first -- weight
            # k-tile t (256 KB, 2 KB lines) interleaved with slab-0 x tile
            # pairs (256 KB, 2 KB lines) in exactly the order slab 0's
            # t-outer loop consumes -- then slabs 1-7 (2 MB, 16 KB lines
            # each): slab 1 lands ~23 us, first needed ~41 us.
            for t in range(KT):
                nc.sync.dma_start(out=w_sb[:, t * D : (t + 1) * D], in_=wT_v[:, t, :])
                if t % 2 == 0:
                    nc.sync.dma_start(
                        out=x_sb[0][:, t * NT : (t + 2) * NT],
                        in_=xP[:, t * NT : (t + 2) * NT],
                    )
            # Slabs 1-4 complete the upfront queue (~14 MB total, drained by
            # ~40 us). Slabs 5-7 (needed from ~100 us) are issued from inside
            # the compute stream instead -- queued that early they would sit
            # AHEAD of slab 0-2's store DMAs in the queue FIFO and starve the
            # o_sb pool (store completion gates o-buf reuse, which gates DVE
            # eviction, which gates PSUM group start: traced as an 8.5 us PE
            # stall + HAM re-throttle at ~56 us in v3).
            for s in range(1, 5):
                load_slab(nc.sync, s)

            def mm(ps, j, t, s, start, stop):
                nc.tensor.matmul(
                    ps,
                    w_sb[:, t * D + j * P : t * D + (j + 1) * P],
                    x_sb[s][:, t * NT : (t + 1) * NT],
                    start=start,
                    stop=stop,
                )

            def evict(ps, j, s, chunks=1):
                o_sb = opool.tile([P, NT], FP32, tag="o", name="o_sb")
                cw = NT // chunks
                for c in range(chunks):
                    sl = slice(c * cw, (c + 1) * cw)
                    nc.vector.tensor_scalar_add(
                        o_sb[:, sl], ps[:, sl], bias_sb[:, j : j + 1]
                    )
                    nc.sync.dma_start(
                        out=yT_v[j, :, s * NT + c * cw : s * NT + (c + 1) * cw],
                        in_=o_sb[:, sl],
                    )

            # Slab 0: t-outer with all 8 PSUM groups open -- each weight
            # k-tile unlocks 8 matmuls, PE streams during the weight load.
            ps0 = [psum_pool.tile([P, NT], FP32, tag="ps", name="ps") for _ in range(JT)]
            for t in range(KT):
                for j in range(JT):
                    mm(ps0[j], j, t, 0, start=(t == 0), stop=(t == KT - 1))
            for j in range(JT):
                evict(ps0[j], j, 0)

            # Slabs 1-7: j-outer, one group per 16-matmul accumulation chain.
            # Late slab loads interleave here: after slab s's evictions, the
            # pending store dma_starts ahead of the load resolve within a few
            # us, so the load issues promptly and lands ~50 us before use.
            for s in range(1, ST):
                if s <= 3:
                    load_slab(nc.sync, s + 4)
                for j in range(JT):
                    ps = psum_pool.tile([P, NT], FP32, tag="ps", name="ps")
                    for t in range(KT):
                        mm(ps, j, t, s, start=(t == 0), stop=(t == KT - 1))
                    last = s == ST - 1 and j == JT - 1
                    evict(ps, j, s, chunks=4 if last else 1)
    nc.finalize()
    return nc


def kernel(feat_rgb, feat_depth, W_k1, b_k1, W_k2, b_k2, W_f, b_f):
    global LAST_RESULT
    feat_rgb = np.asarray(feat_rgb, dtype=np.float32)
    feat_depth = np.asarray(feat_depth, dtype=np.float32)
    W_f = np.asarray(W_f, dtype=np.float32)
    b_f = np.asarray(b_f, dtype=np.float32)

    WfT = np.ascontiguousarray(W_f.T).astype(NP_BF16)  # [2048, 1024]
    biasT = np.ascontiguousarray(b_f.reshape(JT, P).T)  # [128, 8]
    xd = feat_depth.astype(NP_BF16)
    xr = feat_rgb.astype(NP_BF16)

    in_maps = []
    for i in range(NCORES):
        lo, hi = i * BLOC, (i + 1) * BLOC
        x_cat_T = np.empty((K, BLOC), dtype=NP_BF16)
        x_cat_T[:D] = xd[lo:hi].T
        x_cat_T[D:] = xr[lo:hi].T
        # pack to SBUF order [p, s, t, b]: 16 KB contiguous per (p, slab)
        xPk = np.ascontiguousarray(
            x_cat_T.reshape(KT, P, ST, NT).transpose(1, 2, 0, 3).reshape(P, -1)
        )
        in_maps.append({"xP": xPk, "wT": WfT, "biasT": biasT})

    nc = _build_nc()
    res = run_bass_kernel_spmd(
        nc, in_maps, list(range(NCORES)), trace=TRACE, tmpdir=TRACE_DIR
    )
    LAST_RESULT = res

    out = np.empty((B, D), dtype=np.float32)
    for i in range(NCORES):
        out[i * BLOC : (i + 1) * BLOC] = res.results[i]["yT"].T
    return out
